# revision 1
# baseline (speedup 1.0000x reference)
"""CategoryAwareDAHEAD Trainium2 kernel (8-core SPMD, data-parallel over ROIs).

Strategy
--------
* ins_features [512,2048,7,7] is sharded 64 rows/core. Each core streams its
  25.7MB shard and pools 7x7 -> per-core features in a feature-transposed
  layout opT [128, 16*64] (feature f = 128*g + p; a fixed permutation of the
  original d index; W1's rows are permuted identically host-side so the EA
  branch is unchanged; all MGRM uses are inner products over D, which are
  permutation invariant).
* The sequential threshold-gated EMA prototype scan is restructured:
  th <- accept ? (th+m)/2 : th  ==  th <- max(th, (th+m)/2)  (monotone), run
  per-class in parallel over [21, 2S] tables (dom-major along the free dim)
  of per-class sample sequences built with onehot/prefix-sum matmuls.  The
  final prototype is a weighted sum of accepted samples with weight
  w = 1/(j*(j+1)*...*k)  (j = accept index, k = per-class accept count),
  computed as exp(-suffix_sum(A*ln(J))) via matmuls with triangular matrices.
* Per-core partial prototypes / bwl sums / loss_ea partials are AllReduced
  on-device ([64,2048] f32 payload); every core finishes the tiny cosine /
  loss math redundantly and writes the [1,2] result.

KLEVEL env (debug bisection): pool|ea|mgrm1..mgrm4|mgrm|nocc|full
"""

import os
import sys

for _p in ("/opt/trn_rl_repo", "/root/.axon_site/_ro/trn_rl_repo"):
    if _p not in sys.path:
        sys.path.insert(0, _p)

import numpy as np

import concourse.bacc as bacc
import concourse.mybir as mybir
import concourse.tile as tile
from concourse import bass_utils
from concourse.masks import make_identity, make_upper_triangular, make_lower_triangular

F32 = mybir.dt.float32
I32 = mybir.dt.int32
AX = mybir.AxisListType
OP = mybir.AluOpType
ACT = mybir.ActivationFunctionType

NCORES = 8
N, NS, C, D = 512, 256, 21, 2048
NL = N // NCORES          # 64 rows per core
H1 = 1024
DIN = D + C               # 2069
S = 32                    # per-class sequence table length (max count is 18)
GN = 8                    # samples per pooling DMA
THR0, MOM, EPS, LN_EPS, MGRM_W = 0.1, 0.5, 1e-8, 1e-5, 1.0

# feature permutation: new index f holds original d = (f % 128) * 16 + f // 128
PERM = (np.arange(D) % 128) * 16 + np.arange(D) // 128


def _build():
    lv = os.environ.get("KLEVEL", "full")
    sub = {"pool": 0, "ea": 0, "mgrm1": 1, "mgrm2": 2, "mgrm3": 3,
           "mgrm4": 4}.get(lv, 99)
    do_mgrm = lv.startswith("mgrm") or lv in ("nocc", "full")
    do_ea = lv in ("ea", "nocc", "full")
    do_final = lv in ("nocc", "full")

    nc = bacc.Bacc("TRN2", target_bir_lowering=False, debug=False,
                   num_devices=NCORES)

    # ---------------- DRAM I/O ----------------
    feat_dr = nc.dram_tensor("feat_shard", [NL, D, 7, 7], F32, kind="ExternalInput")
    lg_dr = nc.dram_tensor("logits_full", [N, C], F32, kind="ExternalInput")
    lgT_dr = nc.dram_tensor("logitsT_loc", [C, NL], F32, kind="ExternalInput")
    lab_dr = nc.dram_tensor("labels_in", [NS], I32, kind="ExternalInput")
    dom_dr = nc.dram_tensor("dom_shard", [NL, 1], I32, kind="ExternalInput")
    sels_dr = nc.dram_tensor("sel_src", [NS, NL], F32, kind="ExternalInput")
    selt_dr = nc.dram_tensor("sel_tgt", [NS, NL], F32, kind="ExternalInput")
    w1_dr = nc.dram_tensor("W1p", [DIN, H1], F32, kind="ExternalInput")
    w2_dr = nc.dram_tensor("W2in", [H1, H1], F32, kind="ExternalInput")
    w3_dr = nc.dram_tensor("W3in", [H1, H1], F32, kind="ExternalInput")
    wd_dr = nc.dram_tensor("Wdin", [H1, 1], F32, kind="ExternalInput")
    b1_dr = nc.dram_tensor("b1in", [1, H1], F32, kind="ExternalInput")
    b2_dr = nc.dram_tensor("b2in", [1, H1], F32, kind="ExternalInput")
    b3_dr = nc.dram_tensor("b3in", [1, H1], F32, kind="ExternalInput")
    bd_dr = nc.dram_tensor("bdin", [1, 1], F32, kind="ExternalInput")
    out_dr = nc.dram_tensor("out_loss", [1, 2], F32, kind="ExternalOutput")

    with tile.TileContext(nc) as tc:
        with (
            tc.tile_pool(name="consts", bufs=1) as cst,
            tc.tile_pool(name="insb", bufs=1) as insb,
            tc.tile_pool(name="featp", bufs=2) as featp,
            tc.tile_pool(name="persist", bufs=1) as per,
            tc.tile_pool(name="wpool", bufs=4) as wp,
            tc.tile_pool(name="work", bufs=1) as wk,
            tc.tile_pool(name="pps", bufs=3, space="PSUM") as pps,
            tc.tile_pool(name="pph", bufs=2, space="PSUM") as pph,
            tc.tile_pool(name="dram", bufs=1, space="DRAM") as drp,
        ):
            # ---------------- constants ----------------
            id128 = cst.tile([128, 128], F32, tag="id128")
            make_identity(nc, id128[:])
            ut128 = cst.tile([128, 128], F32, tag="ut128")
            make_upper_triangular(nc, ut128[:], val=1.0, diag=True)
            lt128 = cst.tile([128, 128], F32, tag="lt128")
            make_lower_triangular(nc, lt128[:], val=1.0, diag=True)
            ones = cst.tile([128, 128], F32, tag="ones")
            nc.gpsimd.memset(ones[:], 1.0)
            iotaS = cst.tile([128, S], F32, tag="iotaS")   # 1..S per partition
            nc.gpsimd.iota(iotaS[:], [[1, S]], base=1, channel_multiplier=0,
                           allow_small_or_imprecise_dtypes=True)
            iota21 = cst.tile([128, C], F32, tag="iota21")  # 0..20
            nc.gpsimd.iota(iota21[:], [[1, C]], base=0, channel_multiplier=0,
                           allow_small_or_imprecise_dtypes=True)
            epsln = cst.tile([128, 1], F32, tag="epsln")
            nc.gpsimd.memset(epsln[:], LN_EPS)

            # ---------------- small input DMAs ----------------
            lg_sb = insb.tile([128, 4 * C], F32, tag="lg")       # [128, 84]
            nc.scalar.dma_start(
                lg_sb[:].rearrange("p (c l) -> p c l", c=4),
                lg_dr.ap().rearrange("(c p) l -> p c l", p=128))
            lgT_sb = insb.tile([C, NL], F32, tag="lgT")
            nc.scalar.dma_start(lgT_sb[:], lgT_dr[:, :])
            lab_i = insb.tile([128, 2], I32, tag="labi")
            nc.scalar.dma_start(lab_i[:], lab_dr.ap().rearrange(
                "(c p) -> p c", p=128))
            dom_i = insb.tile([NL, 1], I32, tag="domi")
            nc.scalar.dma_start(dom_i[:], dom_dr[:, :])
            sels_sb = insb.tile([128, 2 * NL], F32, tag="sels")
            nc.scalar.dma_start(
                sels_sb[:].rearrange("p (c n) -> p c n", c=2),
                sels_dr.ap().rearrange("(c p) n -> p c n", p=128))
            selt_sb = insb.tile([128, 2 * NL], F32, tag="selt")
            nc.scalar.dma_start(
                selt_sb[:].rearrange("p (c n) -> p c n", c=2),
                selt_dr.ap().rearrange("(c p) n -> p c n", p=128))
            b1_sb = insb.tile([1, H1], F32, tag="b1")
            nc.scalar.dma_start(b1_sb[:], b1_dr[:, :])
            b2_sb = insb.tile([1, H1], F32, tag="b2")
            nc.scalar.dma_start(b2_sb[:], b2_dr[:, :])
            b3_sb = insb.tile([1, H1], F32, tag="b3")
            nc.scalar.dma_start(b3_sb[:], b3_dr[:, :])
            bd_sb = insb.tile([1, 1], F32, tag="bd")
            nc.scalar.dma_start(bd_sb[:], bd_dr[:, :])
            wd_sb = insb.tile([128, 8], F32, tag="wd")
            nc.scalar.dma_start(
                wd_sb[:].rearrange("p (c o) -> p c o", c=8),
                wd_dr.ap().rearrange("(c p) o -> p c o", p=128))

            lab_f = insb.tile([128, 2], F32, tag="labf")
            nc.vector.tensor_copy(lab_f[:], lab_i[:])
            dom_f = insb.tile([NL, 1], F32, tag="domf")
            nc.vector.tensor_copy(dom_f[:], dom_i[:])

            # ---------------- pooling: stream features ----------------
            # feat_dr [NL, 2048, 7, 7] viewed as [128(p), NL, 784] where
            # partition p holds original d in [16p, 16p+16), 784 = 16*49.
            feat_ap = feat_dr.ap().rearrange("n (p g) h w -> p n (g h w)", p=128)
            opT = per.tile([128, 16 * NL], F32, tag="opT")   # f = 128*g+p major
            opT3 = opT[:].rearrange("p (g n) -> p g n", g=16)
            for n0 in range(0, NL, GN):
                ft = featp.tile([128, GN * 784], F32, tag="ft")
                nc.sync.dma_start(ft[:], feat_ap[:, n0:n0 + GN, :])
                red_out = opT3[:, :, n0:n0 + GN].rearrange("p g n -> p n g")
                nc.vector.tensor_reduce(
                    red_out, ft[:].rearrange("p (n g w) -> p n g w", n=GN, g=16),
                    axis=AX.X, op=OP.add)
            # scale by 1/49 (mean) in place
            nc.vector.tensor_scalar_mul(opT[:], opT[:], 1.0 / 49.0)

            # feat64 [64, 2048] sample-major (PE transpose of opT blocks)
            feat64 = per.tile([NL, D], F32, tag="feat64")
            for half in range(2):
                tp = pph.tile([NL, 1024], F32, tag="pph")
                for g in range(8):
                    gg = half * 8 + g
                    nc.tensor.transpose(tp[:, 128 * g:128 * (g + 1)],
                                        opT[:, NL * gg:NL * (gg + 1)],
                                        id128[:])
                nc.scalar.copy(feat64[:, 1024 * half:1024 * (half + 1)], tp[:])

            dbg = None
            counts = None
            bounce = None
            lea_p = None

            if do_mgrm:
                # ---------------- per-chunk softmax stats ----------------
                E_ch, OHP_ch, mlOHP_ch = [], [], []
                for ch in range(4):
                    lg_c = lg_sb[:, C * ch:C * (ch + 1)]
                    mx = wk.tile([128, 1], F32, tag=f"mx{ch}")
                    nc.vector.tensor_reduce(mx[:], lg_c, axis=AX.X, op=OP.max)
                    E = wk.tile([128, C], F32, tag=f"E{ch}")
                    nc.vector.tensor_scalar(out=E[:], in0=lg_c, scalar1=mx[:],
                                            scalar2=None, op0=OP.is_equal)
                    negmx = wk.tile([128, 1], F32, tag=f"nmx{ch}")
                    nc.vector.tensor_scalar_mul(negmx[:], mx[:], -1.0)
                    scr = wk.tile([128, C], F32, tag=f"scr{ch}")
                    den = wk.tile([128, 1], F32, tag=f"den{ch}")
                    nc.scalar.activation(scr[:], lg_c, ACT.Exp, bias=negmx[:],
                                         scale=1.0, accum_out=den[:])
                    ml = wk.tile([128, 1], F32, tag=f"ml{ch}")
                    nc.vector.reciprocal(ml[:], den[:])
                    E_ch.append(E)

                    dom = ch // 2   # 0 = src, 1 = tgt
                    P_ps = pps.tile([128, C], F32, tag="pps")
                    if ch % 2 == 0:
                        nc.tensor.matmul(P_ps[:], ut128[:], E[:], start=True,
                                         stop=True)
                    else:
                        nc.tensor.matmul(P_ps[:], ones[:], E_ch[2 * dom][:],
                                         start=True, stop=False)
                        nc.tensor.matmul(P_ps[:], ut128[:], E[:], start=False,
                                         stop=True)
                    pos = wk.tile([128, 1], F32, tag=f"pos{ch}")
                    posscr = wk.tile([128, C], F32, tag=f"poss{ch}")
                    nc.vector.tensor_tensor(posscr[:], P_ps[:], E[:],
                                            op=OP.mult)
                    nc.vector.tensor_reduce(pos[:], posscr[:], axis=AX.X,
                                            op=OP.add)
                    OHP = wk.tile([128, S], F32, tag=f"OHP{ch}")
                    nc.vector.tensor_scalar(out=OHP[:], in0=iotaS[:],
                                            scalar1=pos[:], scalar2=None,
                                            op0=OP.is_equal)
                    mlOHP = wk.tile([128, S], F32, tag=f"mlO{ch}")
                    nc.vector.tensor_scalar(out=mlOHP[:], in0=iotaS[:],
                                            scalar1=pos[:], scalar2=ml[:],
                                            op0=OP.is_equal, op1=OP.mult)
                    OHP_ch.append(OHP)
                    mlOHP_ch.append(mlOHP)
                dbg = OHP_ch[0]

            if do_mgrm and sub >= 2:
                # tables T[21, 2S]: cols 0..S-1 src, S..2S-1 tgt (dom-major)
                T_all = wk.tile([C, 2 * S], F32, tag="Tall")
                for dom in range(2):
                    T_ps = pps.tile([C, S], F32, tag="pps")
                    nc.tensor.matmul(T_ps[:], E_ch[2 * dom][:],
                                     mlOHP_ch[2 * dom][:], start=True,
                                     stop=False)
                    nc.tensor.matmul(T_ps[:], E_ch[2 * dom + 1][:],
                                     mlOHP_ch[2 * dom + 1][:], start=False,
                                     stop=True)
                    nc.scalar.copy(T_all[:, S * dom:S * (dom + 1)], T_ps[:])

                # ------------- sequential threshold chain -------------
                T3 = T_all[:].rearrange("p (d s) -> p d s", d=2)
                th = wk.tile([C, 2], F32, tag="th")
                nc.gpsimd.memset(th[:], THR0)
                A = wk.tile([C, 2 * S], F32, tag="A")
                A3 = A[:].rearrange("p (d s) -> p d s", d=2)
                tmp = wk.tile([C, 2], F32, tag="chtmp")
                for s in range(S):
                    m = T3[:, :, s]
                    nc.vector.tensor_tensor(A3[:, :, s], m, th[:], op=OP.is_ge)
                    nc.vector.tensor_tensor(tmp[:], m, th[:], op=OP.add)
                    nc.vector.scalar_tensor_tensor(
                        out=th[:], in0=tmp[:], scalar=0.5, in1=th[:],
                        op0=OP.mult, op1=OP.max)
                dbg = A

            if do_mgrm and sub >= 3:
                # per-domain accept-index math -> wtab [21, 2S]
                wtab = wk.tile([C, 2 * S], F32, tag="wtab")
                for dom in range(2):
                    A_dom = A[:, S * dom:S * (dom + 1)]
                    A_T = wk.tile([S, C], F32, tag=f"AT{dom}")
                    at_ps = pps.tile([S, C], F32, tag="pps")
                    nc.tensor.transpose(at_ps[:], A_dom, id128[0:C, 0:C])
                    nc.scalar.copy(A_T[:], at_ps[:])
                    J_ps = pps.tile([C, S], F32, tag="pps")
                    nc.tensor.matmul(J_ps[:], A_T[:], ut128[0:S, 0:S],
                                     start=True, stop=True)
                    jc = wk.tile([C, S], F32, tag=f"jc{dom}")
                    nc.vector.tensor_scalar(out=jc[:], in0=J_ps[:], scalar1=1.0,
                                            scalar2=None, op0=OP.max)
                    lnJ = wk.tile([C, S], F32, tag=f"lnJ{dom}")
                    nc.scalar.activation(lnJ[:], jc[:], ACT.Ln)
                    nc.vector.tensor_tensor(lnJ[:], lnJ[:], A_dom, op=OP.mult)
                    lnJ_T = wk.tile([S, C], F32, tag=f"lnJT{dom}")
                    lt_ps = pps.tile([S, C], F32, tag="pps")
                    nc.tensor.transpose(lt_ps[:], lnJ[:], id128[0:C, 0:C])
                    nc.scalar.copy(lnJ_T[:], lt_ps[:])
                    SS_ps = pps.tile([C, S], F32, tag="pps")
                    nc.tensor.matmul(SS_ps[:], lnJ_T[:], lt128[0:S, 0:S],
                                     start=True, stop=True)
                    wt_dom = wtab[:, S * dom:S * (dom + 1)]
                    nc.scalar.activation(wt_dom, SS_ps[:], ACT.Exp, scale=-1.0)
                    nc.vector.tensor_tensor(wt_dom, wt_dom, A_dom, op=OP.mult)
                dbg = wtab

            EWl_sb = []
            elab_l = None
            if do_mgrm and sub >= 4:
                # ---------- per-sample weights, local gathers ----------
                for dom in range(2):
                    sel = sels_sb if dom == 0 else selt_sb
                    ewl_ps = pps.tile([NL, C], F32, tag="pps")
                    for cc in range(2):
                        ch = 2 * dom + cc
                        ET = wk.tile([C, 128], F32, tag=f"ET{ch}")
                        et_ps = pps.tile([C, 128], F32, tag="pps")
                        nc.tensor.transpose(et_ps[:], E_ch[ch][:], id128[:])
                        nc.scalar.copy(ET[:], et_ps[:])
                        G_ps = pps.tile([128, S], F32, tag="pps")
                        nc.tensor.matmul(G_ps[:], ET[:],
                                         wtab[:, S * dom:S * (dom + 1)],
                                         start=True, stop=True)
                        ws = wk.tile([128, 1], F32, tag=f"ws{ch}")
                        wscr = wk.tile([128, S], F32, tag=f"wscr{ch}")
                        nc.vector.tensor_tensor(wscr[:], G_ps[:],
                                                OHP_ch[ch][:], op=OP.mult)
                        nc.vector.tensor_reduce(ws[:], wscr[:], axis=AX.X,
                                                op=OP.add)
                        EW = wk.tile([128, C], F32, tag=f"EW{ch}")
                        nc.vector.tensor_scalar(out=EW[:], in0=E_ch[ch][:],
                                                scalar1=ws[:], scalar2=None,
                                                op0=OP.mult)
                        nc.tensor.matmul(ewl_ps[:],
                                         sel[:, NL * cc:NL * (cc + 1)],
                                         EW[:], start=(cc == 0), stop=(cc == 1))
                    ewl = wk.tile([NL, C], F32, tag=f"EWl{dom}")
                    nc.scalar.copy(ewl[:], ewl_ps[:])
                    EWl_sb.append(ewl)

                # labels onehot + counts + local label gather
                elab_l_ps = pps.tile([NL, C], F32, tag="pps")
                cnt_ps = pps.tile([C, 1], F32, tag="pps")
                for cc in range(2):
                    Elab = wk.tile([128, C], F32, tag=f"Elab{cc}")
                    nc.vector.tensor_scalar(out=Elab[:], in0=iota21[:],
                                            scalar1=lab_f[:, cc:cc + 1],
                                            scalar2=None, op0=OP.is_equal)
                    nc.tensor.matmul(cnt_ps[:], Elab[:], ones[:, 0:1],
                                     start=(cc == 0), stop=(cc == 1))
                    nc.tensor.matmul(elab_l_ps[:],
                                     sels_sb[:, NL * cc:NL * (cc + 1)],
                                     Elab[:], start=(cc == 0), stop=(cc == 1))
                elab_l = wk.tile([NL, C], F32, tag="elabl")
                nc.scalar.copy(elab_l[:], elab_l_ps[:])
                counts = wk.tile([C, 1], F32, tag="counts")
                nc.scalar.copy(counts[:], cnt_ps[:])
                dbg = counts

            if do_mgrm and sub >= 5:
                # -------- partial protos / bwl -> DRAM bounce --------
                bounce = drp.tile([64, D], F32, tag="bounce")
                for row0, lhs in ((0, EWl_sb[0]), (C, EWl_sb[1]),
                                  (2 * C, elab_l)):
                    pr_sb = wk.tile([C, D], F32, tag="prsb")
                    for j in range(4):
                        pr_ps = pps.tile([C, 512], F32, tag="pps")
                        nc.tensor.matmul(pr_ps[:], lhs[:],
                                         feat64[:, 512 * j:512 * (j + 1)],
                                         start=True, stop=True)
                        nc.scalar.copy(pr_sb[:, 512 * j:512 * (j + 1)],
                                       pr_ps[:])
                    nc.sync.dma_start(bounce[row0:row0 + C, :], pr_sb[:])

            if do_ea:
                # ---------------- EA branch (per-core rows) ----------------
                def ln_relu(h_ps):
                    musum = wk.tile([NL, 1], F32, tag="mu")
                    nc.vector.tensor_reduce(musum[:], h_ps[:], axis=AX.X,
                                            op=OP.add)
                    mu = wk.tile([NL, 1], F32, tag="mus")
                    nc.vector.tensor_scalar_mul(mu[:], musum[:], 1.0 / H1)
                    xc = wk.tile([NL, H1], F32, tag="xc")
                    nc.vector.tensor_scalar(out=xc[:], in0=h_ps[:],
                                            scalar1=mu[:], scalar2=None,
                                            op0=OP.subtract)
                    vscr = wk.tile([NL, H1], F32, tag="vs")
                    vsum = wk.tile([NL, 1], F32, tag="v")
                    nc.scalar.activation(vscr[:], xc[:], ACT.Square,
                                         accum_out=vsum[:])
                    sd = wk.tile([NL, 1], F32, tag="sd")
                    nc.scalar.activation(sd[:], vsum[:], ACT.Sqrt,
                                         scale=1.0 / H1, bias=epsln[0:NL, :])
                    rstd = wk.tile([NL, 1], F32, tag="rs")
                    nc.vector.reciprocal(rstd[:], sd[:])
                    h = wk.tile([NL, H1], F32, tag="h")
                    nc.vector.tensor_scalar(out=h[:], in0=xc[:],
                                            scalar1=rstd[:], scalar2=0.0,
                                            op0=OP.mult, op1=OP.max)
                    return h

                # hT [128, 8*64]: block j holds features 128j..128j+127
                def transpose_h2(h):
                    hT = wk.tile([128, 8 * NL], F32, tag="hT")
                    for half in range(2):
                        ht_ps = pps.tile([128, 4 * NL], F32, tag="pps")
                        for j in range(4):
                            jj = 4 * half + j
                            nc.tensor.transpose(ht_ps[:, NL * j:NL * (j + 1)],
                                                h[:, 128 * jj:128 * (jj + 1)],
                                                id128[0:NL, 0:NL])
                        nc.scalar.copy(
                            hT[:, 4 * NL * half:4 * NL * (half + 1)], ht_ps[:])
                    return hT

                # layer 1: lhsT chunks = opT blocks + logitsT
                h_ps = pph.tile([NL, H1], F32, tag="pph")
                for half in range(2):
                    nsl = slice(512 * half, 512 * (half + 1))
                    nc.tensor.matmul(h_ps[:, nsl], ones[0:1, 0:NL],
                                     b1_sb[:, nsl], start=True, stop=False)
                for kc in range(17):
                    kp = 128 if kc < 16 else C
                    wt = wp.tile([128, H1], F32, tag="w")
                    nc.scalar.dma_start(wt[0:kp, :],
                                        w1_dr[128 * kc:128 * kc + kp, :])
                    lhsT = (opT[:, NL * kc:NL * (kc + 1)] if kc < 16
                            else lgT_sb[:, :])
                    for half in range(2):
                        nsl = slice(512 * half, 512 * (half + 1))
                        nc.tensor.matmul(h_ps[:, nsl], lhsT, wt[0:kp, nsl],
                                         start=False, stop=(kc == 16))
                h = ln_relu(h_ps)

                for w_dr, b_sb in ((w2_dr, b2_sb), (w3_dr, b3_sb)):
                    hT = transpose_h2(h)
                    h_ps = pph.tile([NL, H1], F32, tag="pph")
                    for half in range(2):
                        nsl = slice(512 * half, 512 * (half + 1))
                        nc.tensor.matmul(h_ps[:, nsl], ones[0:1, 0:NL],
                                         b_sb[:, nsl], start=True, stop=False)
                    for kc in range(8):
                        wt = wp.tile([128, H1], F32, tag="w")
                        nc.scalar.dma_start(wt[:],
                                            w_dr[128 * kc:128 * (kc + 1), :])
                        for half in range(2):
                            nsl = slice(512 * half, 512 * (half + 1))
                            nc.tensor.matmul(h_ps[:, nsl],
                                             hT[:, NL * kc:NL * (kc + 1)],
                                             wt[:, nsl],
                                             start=False, stop=(kc == 7))
                    h = ln_relu(h_ps)

                h3T = transpose_h2(h)
                zd_ps = pps.tile([NL, 1], F32, tag="pps")
                nc.tensor.matmul(zd_ps[:], ones[0:1, 0:NL], bd_sb[:],
                                 start=True, stop=False)
                for kc in range(8):
                    nc.tensor.matmul(zd_ps[:], h3T[:, NL * kc:NL * (kc + 1)],
                                     wd_sb[:, kc:kc + 1], start=False,
                                     stop=(kc == 7))
                z = wk.tile([NL, 1], F32, tag="z")
                nc.scalar.activation(z[:], zd_ps[:], ACT.Sigmoid)
                # softplus(-z) = ln(1 + exp(-z))  (Softplus has no ACT table)
                enz = wk.tile([NL, 1], F32, tag="enz")
                nc.scalar.activation(enz[:], z[:], ACT.Exp, scale=-1.0)
                sp = wk.tile([NL, 1], F32, tag="sp")
                nc.scalar.activation(sp[:], enz[:], ACT.Ln,
                                     bias=ones[0:NL, 0:1])
                omy = wk.tile([NL, 1], F32, tag="omy")
                nc.vector.tensor_scalar(out=omy[:], in0=dom_f[:], scalar1=-1.0,
                                        scalar2=1.0, op0=OP.mult, op1=OP.add)
                li_t = wk.tile([NL, 1], F32, tag="li")
                nc.vector.scalar_tensor_tensor(out=li_t[:], in0=z[:],
                                               scalar=omy[:], in1=sp[:],
                                               op0=OP.mult, op1=OP.add)
                lea_ps = pps.tile([1, 1], F32, tag="pps")
                nc.tensor.matmul(lea_ps[:], li_t[:], ones[0:NL, 0:1],
                                 start=True, stop=True)
                lea_p = wk.tile([1, 1], F32, tag="leap")
                nc.scalar.copy(lea_p[:], lea_ps[:])
                if do_mgrm and sub >= 5:
                    lea_row = wk.tile([1, D], F32, tag="learow")
                    nc.gpsimd.memset(lea_row[:], 0.0)
                    nc.scalar.copy(lea_row[:, 0:1], lea_ps[:])
                    nc.sync.dma_start(bounce[63:64, :], lea_row[:])

            if do_final:
                # ---------------- AllReduce ----------------
                bounce_out = drp.tile([64, D], F32, tag="bounce_out")
                if lv == "nocc":
                    nc.sync.dma_start(bounce_out[:], bounce[:])
                else:
                    nc.gpsimd.collective_compute(
                        "AllReduce", OP.add,
                        replica_groups=[list(range(NCORES))],
                        ins=[bounce[:].opt()], outs=[bounce_out[:].opt()])
                ps_sb = per.tile([C, D], F32, tag="psr")
                nc.sync.dma_start(ps_sb[:], bounce_out[0:C, :])
                pt_sb = per.tile([C, D], F32, tag="ptr")
                nc.sync.dma_start(pt_sb[:], bounce_out[C:2 * C, :])
                bw_sb = per.tile([C, D], F32, tag="bwr")
                nc.sync.dma_start(bw_sb[:], bounce_out[2 * C:3 * C, :])
                lea_sb = wk.tile([1, 1], F32, tag="leas")
                nc.sync.dma_start(lea_sb[:], bounce_out[63:64, 0:1])

                # ---------------- final (replicated) ----------------
                ps_r, pt_r, bw_r = ps_sb[:], pt_sb[:], bw_sb[:]
                cexist = wk.tile([C, 1], F32, tag="cex")
                nc.vector.tensor_scalar(out=cexist[:], in0=counts[:],
                                        scalar1=0.0, scalar2=None,
                                        op0=OP.is_gt)
                cmax = wk.tile([C, 1], F32, tag="cmax")
                nc.vector.tensor_scalar(out=cmax[:], in0=counts[:], scalar1=1.0,
                                        scalar2=None, op0=OP.max)
                crec = wk.tile([C, 1], F32, tag="crec")
                nc.vector.reciprocal(crec[:], cmax[:])

                rns = []
                for idx, srcm in enumerate((ps_r, pt_r, bw_r)):
                    nsq = wk.tile([C, 1], F32, tag=f"nsq{idx}")
                    nscr = wk.tile([C, D], F32, tag="nscr")
                    nc.scalar.activation(nscr[:], srcm, ACT.Square,
                                         accum_out=nsq[:])
                    nrm = wk.tile([C, 1], F32, tag=f"nrm{idx}")
                    nc.scalar.activation(nrm[:], nsq[:], ACT.Sqrt)
                    if idx == 2:  # bwl norm = |sums| * crec
                        nc.vector.tensor_scalar(out=nrm[:], in0=nrm[:],
                                                scalar1=crec[:],
                                                scalar2=float(EPS),
                                                op0=OP.mult, op1=OP.max)
                    else:
                        nc.vector.tensor_scalar(out=nrm[:], in0=nrm[:],
                                                scalar1=float(EPS),
                                                scalar2=None, op0=OP.max)
                    rn = wk.tile([C, 1], F32, tag=f"rn{idx}")
                    nc.vector.reciprocal(rn[:], nrm[:])
                    rns.append(rn)
                # bwl rows also carry the 1/count factor
                sc_bw = wk.tile([C, 1], F32, tag="scbw")
                nc.vector.tensor_tensor(sc_bw[:], rns[2][:], crec[:],
                                        op=OP.mult)

                normed = []
                for srcm, sc in ((ps_r, rns[0]), (pt_r, rns[1]),
                                 (bw_r, sc_bw)):
                    nc.vector.tensor_scalar(out=srcm, in0=srcm, scalar1=sc[:],
                                            scalar2=None, op0=OP.mult)
                    normed.append(srcm)

                # transpose to [128, 16*21] chunks for cosine matmuls
                nT = []
                for idx, srcm in enumerate(normed):
                    xt = wk.tile([128, 16 * C], F32, tag=f"xt{idx}")
                    xt_ps = pps.tile([128, 16 * C], F32, tag="pps")
                    for t in range(16):
                        nc.tensor.transpose(xt_ps[:, C * t:C * (t + 1)],
                                            srcm[:, 128 * t:128 * (t + 1)],
                                            id128[0:C, 0:C])
                    nc.scalar.copy(xt[:], xt_ps[:])
                    nT.append(xt)

                ptm_ps = pps.tile([C, C], F32, tag="pps")
                btm_ps = pps.tile([C, C], F32, tag="pps")
                for t in range(16):
                    tsl = slice(C * t, C * (t + 1))
                    nc.tensor.matmul(ptm_ps[:], nT[1][:, tsl], nT[0][:, tsl],
                                     start=(t == 0), stop=(t == 15))
                    nc.tensor.matmul(btm_ps[:], nT[1][:, tsl], nT[2][:, tsl],
                                     start=(t == 0), stop=(t == 15))
                ptm_sb = wk.tile([C, C], F32, tag="ptmsb")
                nc.scalar.copy(ptm_sb[:], ptm_ps[:])
                dsb = wk.tile([C, C], F32, tag="dsb")
                nc.vector.tensor_tensor(dsb[:], btm_ps[:], ptm_sb[:],
                                        op=OP.subtract)
                rowd = wk.tile([C, 1], F32, tag="rowd")
                nc.vector.tensor_reduce(rowd[:], dsb[:, 1:C], axis=AX.X,
                                        op=OP.add, apply_absolute_value=True)
                # cem = exist mask with class 0 zeroed (drops row 0)
                cem = wk.tile([C, 1], F32, tag="cem")
                nc.vector.tensor_copy(cem[:], cexist[:])
                nc.gpsimd.memset(cem[0:1, :], 0.0)
                tot_ps = pps.tile([1, 1], F32, tag="pps")
                nc.tensor.matmul(tot_ps[:], rowd[:], cem[:], start=True,
                                 stop=True)
                nm_ps = pps.tile([1, 1], F32, tag="pps")
                nc.tensor.matmul(nm_ps[:], cem[:], ones[0:C, 0:1],
                                 start=True, stop=True)
                nm_sb = wk.tile([1, 1], F32, tag="nmsb")
                nc.scalar.copy(nm_sb[:], nm_ps[:])
                rnm = wk.tile([1, 1], F32, tag="rnm")
                nc.vector.reciprocal(rnm[:], nm_sb[:])

                res = wk.tile([1, 2], F32, tag="res")
                nc.vector.tensor_scalar(out=res[:, 0:1], in0=tot_ps[:],
                                        scalar1=rnm[:],
                                        scalar2=MGRM_W / (C - 1.0),
                                        op0=OP.mult, op1=OP.mult)
                nc.vector.tensor_scalar(out=res[:, 1:2], in0=lea_sb[:],
                                        scalar1=1.0 / N, scalar2=None,
                                        op0=OP.mult)
                nc.sync.dma_start(out_dr[:, :], res[:])
            else:
                res = wk.tile([1, 2], F32, tag="res")
                if lv == "pool":
                    nc.scalar.copy(res[:], feat64[0:1, 0:2])
                elif lv == "ea":
                    nc.scalar.copy(res[:, 0:1], lea_p[:])
                    nc.scalar.copy(res[:, 1:2], feat64[0:1, 0:1])
                else:
                    nc.scalar.copy(res[:, 0:1], dbg[0:1, 0:1])
                    nc.scalar.copy(res[:, 1:2], feat64[0:1, 0:1])
                nc.sync.dma_start(out_dr[:, :], res[:])

    nc.compile()
    return nc


_NC_CACHE = {}
_last_in_maps = None


def _prep_in_maps(inputs):
    feats = np.ascontiguousarray(inputs["ins_features"], dtype=np.float32)
    logits = np.ascontiguousarray(inputs["class_logits"], dtype=np.float32)
    labels = np.ascontiguousarray(inputs["labels"], dtype=np.int32)
    dom = np.ascontiguousarray(inputs["domain_labels"], dtype=np.int32)
    W1 = np.asarray(inputs["W1"], np.float32)
    W1p = np.ascontiguousarray(np.concatenate([W1[:D][PERM], W1[D:]], axis=0))
    W2 = np.ascontiguousarray(inputs["W2"], np.float32)
    W3 = np.ascontiguousarray(inputs["W3"], np.float32)
    Wd = np.ascontiguousarray(inputs["Wd"], np.float32)
    b1 = np.ascontiguousarray(inputs["b1"], np.float32).reshape(1, H1)
    b2 = np.ascontiguousarray(inputs["b2"], np.float32).reshape(1, H1)
    b3 = np.ascontiguousarray(inputs["b3"], np.float32).reshape(1, H1)
    bd = np.ascontiguousarray(inputs["bd"], np.float32).reshape(1, 1)

    in_maps = []
    for k in range(NCORES):
        r0 = NL * k
        sel_s = np.zeros((NS, NL), np.float32)
        sel_t = np.zeros((NS, NL), np.float32)
        if r0 + NL <= NS:
            sel_s[np.arange(r0, r0 + NL), np.arange(NL)] = 1.0
        else:
            sel_t[np.arange(r0 - NS, r0 - NS + NL), np.arange(NL)] = 1.0
        in_maps.append({
            "feat_shard": np.ascontiguousarray(feats[r0:r0 + NL]),
            "logits_full": logits,
            "logitsT_loc": np.ascontiguousarray(logits[r0:r0 + NL].T),
            "labels_in": labels,
            "dom_shard": np.ascontiguousarray(dom[r0:r0 + NL].reshape(NL, 1)),
            "sel_src": sel_s,
            "sel_tgt": sel_t,
            "W1p": W1p, "W2in": W2, "W3in": W3, "Wdin": Wd,
            "b1in": b1, "b2in": b2, "b3in": b3, "bdin": bd,
        })
    return in_maps


def kernel(**inputs) -> np.ndarray:
    if "nc" not in _NC_CACHE:
        _NC_CACHE["nc"] = _build()
    nc = _NC_CACHE["nc"]
    in_maps = _prep_in_maps(inputs)
    global _last_in_maps
    _last_in_maps = in_maps
    res = bass_utils.run_bass_kernel_spmd(nc, in_maps,
                                          core_ids=list(range(NCORES)))
    return res.results[0]["out_loss"].reshape(2).astype(np.float32)

